# revision 1
# baseline (speedup 1.0000x reference)
"""AnchorLoss distributed Trainium2 kernel (8 NeuronCores).

reference math (anchors: [8192, 8, 512] f32):
    x = anchors.reshape(8192, 4096)
    loss = -(2*N*sum(x*x) - 2*sum(colsum(x)^2)) / sqrt(512)

Strategy: shard COLUMNS across the 8 cores (512 columns each). Each core
streams its [8192, 512] column slice (16 MiB) in 16 tiles of
[128, 4x512] (4 row-blocks per tile) over TWO parallel DMA rings:
5 tiles stay f32 on the SP HWDGE ring; 11 tiles are converted
f32->bf16 inside the gpsimd SWDGE DMA (HBM still reads every f32 byte
once - bf16 is an internal compute-precision choice). Per tile:
  - partial sum of squares, load-balanced across ScalarE
    (Square + accum_out) and VectorE (2x bf16 tensor_mul, then a x1.0
    tensor_scalar whose accum_out reduces at 4x)
  - the COMPLETE column sums of its 512 columns via PE matmuls
    (lhsT = x block [128,128], rhs = ones [128,1], PSUM-accumulated
    over the 4 row-blocks, then SBUF-accumulated over tiles)
so the only cross-core data is one scalar per core:
    c_k = (2/f)*||colsum_k||^2 - (2*N/f)*sumsq_k
Each core replicates c_k 8x and a ReduceScatter-add leaves
loss = sum_k c_k (= -total/f) in every core's [1] bounce buffer;
a DRAM->DRAM copy lands it in "out". Host takes core 0's scalar.
"""

import numpy as np

from concourse import bacc, tile, mybir
from concourse.bass_utils import run_bass_kernel_spmd

N_CORES = 8
N_CLASSES = 8192
D = 4096                        # 8 * 512 flattened embedding dim
COLS = D // N_CORES             # 512 columns per core
P = 128                         # partitions
RB = 4                          # row-blocks per tile
TILE_ROWS = P * RB              # 512 rows per tile
N_TILES = N_CLASSES // TILE_ROWS  # 16
CHUNK = 128                     # columns per colsum matmul
N_CHUNKS = COLS // CHUNK        # 4
FACTOR = float(np.sqrt(np.float32(512.0)))


def _build():
    nc = bacc.Bacc(None, num_devices=N_CORES)
    x_ext = nc.declare_dram_parameter(
        "anchors", [N_CLASSES, COLS], mybir.dt.float32, isOutput=False
    )
    out_ext = nc.declare_dram_parameter(
        "out", [1, 1], mybir.dt.float32, isOutput=True
    )

    with tile.TileContext(nc) as tc:
        with (
            tc.tile_pool(name="io", bufs=6) as io,
            tc.tile_pool(name="small", bufs=1) as sp,
            tc.tile_pool(name="psum", bufs=1, space="PSUM") as ps,
            tc.tile_pool(name="dram", bufs=1, space="DRAM") as dr,
        ):
            ones = sp.tile([P, 1], mybir.dt.float32)
            nc.gpsimd.memset(ones[:], 1.0)
            ones_bf = sp.tile([P, 1], mybir.dt.bfloat16)
            nc.gpsimd.memset(ones_bf[:], 1.0)
            # one accum column per (tile, sub-square): the last two tiles
            # split their square into RB chunks to shorten the critical tail
            rowsumsq = sp.tile([P, N_TILES + 2 * (RB - 1)], mybir.dt.float32)
            scr_s = sp.tile([P, RB, COLS], mybir.dt.float32)
            scr_sb = sp.tile([P, RB, COLS], mybir.dt.bfloat16)
            scr_vu = sp.tile([P, COLS], mybir.dt.bfloat16)
            scr_vb = sp.tile([P, RB, COLS], mybir.dt.bfloat16)
            cs_acc = sp.tile([P, N_CHUNKS], mybir.dt.float32)
            nc.vector.memset(cs_acc[:], 0.0)

            for t in range(N_TILES):
                # alternate tiles between the SP HWDGE ring (f32) and the
                # gpsimd SWDGE ring (converted f32->bf16 in the DMA) so the
                # two DMA FIFOs stream in parallel and bf16 tiles square
                # at 2x on ACT/DVE. HBM still reads every f32 byte once.
                bf = t not in (0, 3, 6, 9, 12)
                dt_t = mybir.dt.bfloat16 if bf else mybir.dt.float32
                dma_eng = nc.gpsimd if bf else nc.sync
                one_t = ones_bf if bf else ones
                xt = io.tile([P, RB, COLS], dt_t,
                             tag="xtb" if bf else "xt", name=f"xt{t}")
                src = x_ext[t * TILE_ROWS:(t + 1) * TILE_ROWS, :]
                src = src.rearrange("(rb p) c -> p rb c", rb=RB, p=P)
                # the last two tiles are DMA'd and squared per row-block so
                # only a short square trails the final DMA
                if t < N_TILES - 2:
                    dma_eng.dma_start(xt[:], src)
                    if t in (1, 2, 4, 5, 7, 8, 10, 13):
                        # bf16 full squares on DVE: 2x mult, then a x1.0
                        # tensor_scalar whose accum_out sums at 4x
                        nc.vector.tensor_mul(scr_vb[:], xt[:], xt[:])
                        nc.vector.tensor_scalar(
                            scr_vb[:], scr_vb[:], 1.0, None,
                            mybir.AluOpType.mult, mybir.AluOpType.add,
                            accum_out=rowsumsq[:, t:t + 1],
                        )
                    else:
                        # the rest on ScalarE
                        scr = scr_sb if bf else scr_s
                        nc.scalar.activation(
                            scr[:], xt[:],
                            mybir.ActivationFunctionType.Square,
                            accum_out=rowsumsq[:, t:t + 1],
                        )
                else:
                    base = t + (t - (N_TILES - 2)) * (RB - 1)
                    dma_eng.dma_start(xt[:], src)
                    for j in range(RB):
                        col = rowsumsq[:, base + j:base + j + 1]
                        if (t, j) in ((N_TILES - 2, 0), (N_TILES - 2, 1),
                                      (N_TILES - 2, 3), (N_TILES - 1, 1),
                                      (N_TILES - 1, 3)):
                            # some unit pairs on DVE
                            nc.vector.tensor_mul(scr_vu[:], xt[:, j, :],
                                                 xt[:, j, :])
                            nc.vector.tensor_scalar(
                                scr_vu[:], scr_vu[:], 1.0, None,
                                mybir.AluOpType.mult, mybir.AluOpType.add,
                                accum_out=col,
                            )
                        else:
                            # f32 units j0/j2 and all bf16 units on ScalarE
                            scr = scr_sb if bf else scr_s
                            nc.scalar.activation(
                                scr[:, j, :], xt[:, j, :],
                                mybir.ActivationFunctionType.Square,
                                accum_out=col,
                            )
                # column sums of this tile's 512 rows:
                # cs_ps[m, c] = sum_{rb,p} xt[p, rb, c*128+m]
                cs_ps = ps.tile(
                    [P, N_CHUNKS], mybir.dt.float32, tag="cs_ps",
                    name=f"cs{t}", bufs=2,
                )
                for c in range(N_CHUNKS):
                    for j in range(RB):
                        nc.tensor.matmul(
                            cs_ps[:, c:c + 1],
                            lhsT=xt[:, j, c * CHUNK:(c + 1) * CHUNK],
                            rhs=one_t[:],
                            start=(j == 0), stop=(j == RB - 1),
                        )
                nc.vector.tensor_add(cs_acc[:], cs_acc[:], cs_ps[:])

            # local scalars: F[:,0] = per-partition sumsq, F[:,1] = colsum^2
            F = sp.tile([P, 2], mybir.dt.float32)
            nc.vector.tensor_reduce(
                out=F[:, 0:1], in_=rowsumsq[:],
                axis=mybir.AxisListType.X, op=mybir.AluOpType.add,
            )
            # colsum^2 on DVE (keeps it off ScalarE's tail queue)
            scr2 = sp.tile([P, N_CHUNKS], mybir.dt.float32)
            nc.vector.tensor_mul(scr2[:], cs_acc[:], cs_acc[:])
            nc.vector.tensor_reduce(
                out=F[:, 1:2], in_=scr2[:],
                axis=mybir.AxisListType.X, op=mybir.AluOpType.add,
            )
            res_ps = ps.tile([1, 2], mybir.dt.float32)
            nc.tensor.matmul(res_ps[:], lhsT=ones[:], rhs=F[:],
                             start=True, stop=True)
            # c_k = (2/f)*colsumsq_k - (2*N/f)*sumsq_k
            a_sb = sp.tile([1, 1], mybir.dt.float32)
            nc.vector.tensor_scalar_mul(
                a_sb[:], res_ps[0:1, 0:1], float(2.0 * N_CLASSES / FACTOR)
            )
            ck_sb = sp.tile([1, 1], mybir.dt.float32)
            nc.vector.scalar_tensor_tensor(
                out=ck_sb[:], in0=res_ps[0:1, 1:2],
                scalar=float(2.0 / FACTOR), in1=a_sb[:],
                op0=mybir.AluOpType.mult, op1=mybir.AluOpType.subtract,
            )

            # sum the 8 per-core scalars: replicate ck 8x, ReduceScatter-add
            # -> each core's [1] output IS the loss; copy DRAM->DRAM to out
            ck8 = sp.tile([1, N_CORES], mybir.dt.float32)
            nc.vector.tensor_copy(ck8[:], ck_sb[:].broadcast_to([1, N_CORES]))
            cc_in = dr.tile([N_CORES], mybir.dt.float32)
            cc_out = dr.tile([1], mybir.dt.float32)
            nc.sync.dma_start(cc_in[:], ck8[:])
            nc.gpsimd.collective_compute(
                "ReduceScatter",
                mybir.AluOpType.add,
                replica_groups=[list(range(N_CORES))],
                ins=[cc_in[:]],
                outs=[cc_out[:]],
            )
            nc.sync.dma_start(out_ext[:], cc_out[:])
    nc.finalize()
    return nc


_NC_CACHE = None


def _get_nc():
    global _NC_CACHE
    if _NC_CACHE is None:
        _NC_CACHE = _build()
    return _NC_CACHE


def _run(anchors: np.ndarray, trace: bool = False):
    """Returns (loss_scalar, BassKernelResults)."""
    x = np.asarray(anchors, dtype=np.float32).reshape(N_CLASSES, D)
    in_maps = [
        {"anchors": np.ascontiguousarray(x[:, i * COLS:(i + 1) * COLS])}
        for i in range(N_CORES)
    ]
    nc = _get_nc()
    res = run_bass_kernel_spmd(nc, in_maps, core_ids=list(range(N_CORES)), trace=trace)
    loss = np.float32(np.asarray(res.results[0]["out"]).reshape(())[()])
    return loss, res


def kernel(anchors: np.ndarray) -> np.ndarray:
    loss, _ = _run(anchors)
    return np.asarray(loss, dtype=np.float32).reshape(())



# revision 6
# speedup vs baseline: 2.1237x; 2.1237x over previous
"""AnchorLoss distributed Trainium2 kernel (8 NeuronCores).

reference math (anchors: [8192, 8, 512] f32):
    x = anchors.reshape(8192, 4096)
    loss = -(2*N*sum(x*x) - 2*sum(colsum(x)^2)) / sqrt(512)

Strategy: shard COLUMNS across the 8 cores (512 columns each); every
cross-core quantity is then a single scalar per core. Each core streams
its [8192, 512] f32 slice as 16 row-tiles of [128, 4, 512]:

  - 11 "gram" tiles are cast f32->fp8e4 inside the Pool SWDGE DMA (HBM
    still reads every f32 byte once); PE accumulates per-chunk Gram
    matrices X_c^T X_c into one PSUM bank [128, 4x128] whose diagonal
    is the tile's sum of squares. PE also column-sums every tile via
    ones-vector matmuls into a second PSUM bank [128, 4].
  - 2 tiles are cast f32->bf16 (Pool DMA) and squared on DVE
    (2x tensor_mul + 4x tensor_scalar accumulate).
  - 3 tiles stay f32 on the SP HWDGE ring and are squared on ScalarE.

This splits the elementwise-square roofline across PE/DVE/ACT while
Pool+SP share the DMA bytes, so all five engines run ~balanced.

The per-core partial  c_k = (2/f)*||colsum_k||^2 - (2N/f)*sumsq_k  is
collapsed to a scalar with a ones^T matmul, then summed across cores
WITHOUT the 15us collective: a raw post-tile block runs a single-shot
all-to-all of the 8 scalars via XOR-relative remote_dma_broadcast
(7 single-slot broadcasts, slot d targets core ^ d; the hardware XORs
physical ids, which relabels peers but stays a bijection, so the sum
is invariant). Each core then tree-adds the 8 values and SP DMAs the
total to "out"; the host reads core 0.
"""

import numpy as np
from contextlib import ExitStack

from concourse import bacc, bass, tile, mybir
from concourse.bass_utils import run_bass_kernel_spmd

# The axon client container has no /dev/neuron*, so the driver ioctls
# behind these routing lookups fail. The simulator only needs a sane
# single-device identity mapping (8 cores on device 0); the real NEFF
# resolves XOR-relative routing on-device and never reads these.
import concourse.libnrt as _lnrt
import concourse.bass_interp as _bi
try:
    _lnrt.get_trn2_nc_mapping()
except Exception:
    _IDENT = {(0, i): i for i in range(8)}
    _RID = {0: 0}
    _lnrt.get_trn2_nc_mapping = lambda: _IDENT
    _lnrt.get_device_id_to_routing_id_mapping = lambda: _RID
    _bi.get_device_id_to_routing_id_mapping = lambda: _RID

N_CORES = 8
N_CLASSES = 8192
D = 4096                          # 8 * 512 flattened embedding dim
COLS = D // N_CORES               # 512 columns per core
P = 128                           # partitions
RB = 4                            # row-blocks per tile
TILE_ROWS = P * RB                # 512 rows per tile
N_TILES = N_CLASSES // TILE_ROWS  # 16
CHUNK = 128
N_CHUNKS = COLS // CHUNK          # 4
FACTOR = float(np.sqrt(np.float32(512.0)))

N_GRAM = 11                       # fp8 tiles -> PE gram diag
N_DVE = 2                         # bf16 tiles -> DVE squares
N_ACT = N_TILES - N_GRAM - N_DVE  # f32 tiles (SP DMA) -> ACT squares
PE_WARMUP = 20                    # dummy matmuls to ramp the PE p-state


def _build():
    nc = bacc.Bacc(None, num_devices=N_CORES)
    x_ext = nc.declare_dram_parameter(
        "anchors", [N_CLASSES, COLS], mybir.dt.float32, isOutput=False
    )
    out_ext = nc.declare_dram_parameter(
        "out", [1, 1], mybir.dt.float32, isOutput=True
    )

    es = ExitStack()
    # raw SBUF tensors shared with the post-tile all-to-all block
    recv = es.enter_context(nc.sbuf_tensor("recv8", [P, 8], mybir.dt.float32))
    recvb = es.enter_context(nc.sbuf_tensor("recvb8", [P, 8], mybir.dt.float32))
    radd = es.enter_context(nc.sbuf_tensor("radd", [P, 4], mybir.dt.float32))
    tot = es.enter_context(nc.sbuf_tensor("tot", [P, 1], mybir.dt.float32))
    pad = es.enter_context(nc.sbuf_tensor("pad", [P, 512], mybir.dt.float32))
    lsem = nc.alloc_semaphore("aa_lsem")
    prep_sem = nc.alloc_semaphore("aa_prep")
    dsem = nc.alloc_semaphore("aa_dsem")
    cp_sem = nc.alloc_semaphore("aa_cp")
    out_sem = nc.alloc_semaphore("aa_out")

    def rcol(d):
        return bass.AP(recv, d, [[8, P], [1, 1]])

    with tile.TileContext(nc) as tc:
        with (
            tc.tile_pool(name="io", bufs=6) as io,
            tc.tile_pool(name="small", bufs=1) as sp,
            tc.tile_pool(name="psum", bufs=1, space="PSUM") as ps,
        ):
            # constants (keep Pool free: build on DVE where possible)
            ones8 = sp.tile([P, 1], mybir.dt.float8e4)
            nc.vector.memset(ones8[:], 1.0)
            ones_bf = sp.tile([P, 1], mybir.dt.bfloat16)
            nc.vector.memset(ones_bf[:], 1.0)
            ones_f = sp.tile([P, 1], mybir.dt.float32)
            nc.vector.memset(ones_f[:], 1.0)
            wones = sp.tile([P, CHUNK], mybir.dt.bfloat16)
            nc.vector.memset(wones[:], 0.001)
            nc.vector.memset(bass.AP(recv, 0, [[8, P], [1, 8]]), 0.0)
            nc.vector.memset(bass.AP(pad, 0, [[512, P], [1, 512]]), 0.0)
            # identity mask for the gram diagonal: eye[p, q] = (q == p)
            iq = sp.tile([P, CHUNK], mybir.dt.float32)
            nc.gpsimd.iota(iq[:], [[1, CHUNK]], channel_multiplier=0,
                           allow_small_or_imprecise_dtypes=True)
            ip = sp.tile([P, 1], mybir.dt.float32)
            nc.gpsimd.iota(ip[:], [[0, 1]], channel_multiplier=1,
                           allow_small_or_imprecise_dtypes=True)
            eye = sp.tile([P, CHUNK], mybir.dt.float32)
            nc.vector.tensor_tensor(
                eye[:], iq[:], ip[:].broadcast_to([P, CHUNK]),
                mybir.AluOpType.is_equal)
            eye4 = sp.tile([P, N_CHUNKS, CHUNK], mybir.dt.float32)
            for c in range(N_CHUNKS):
                nc.vector.tensor_copy(eye4[:, c, :], eye[:])

            # ACT table preload: tiny square so LoadActFuncSet runs early
            warm_a = sp.tile([P, 1], mybir.dt.float32)
            nc.scalar.activation(warm_a[:], ones_f[:],
                                 mybir.ActivationFunctionType.Square)

            # PE p-state warmup: dummy matmuls while DMAs stream
            warm_ps = ps.tile([P, CHUNK], mybir.dt.float32)
            for i in range(PE_WARMUP):
                nc.tensor.matmul(warm_ps[:], lhsT=wones[:], rhs=wones[:],
                                 start=True, stop=True)

            # PSUM accumulators: gram bank (4 chunks side by side) and
            # column-sum bank. One zero-region start on the global first
            # matmul, one stop on the global last.
            gram = ps.tile([P, N_CHUNKS * CHUNK], mybir.dt.float32)
            cs = ps.tile([P, N_CHUNKS], mybir.dt.float32)

            # accumulator columns for DVE/ACT tile row-sums of squares
            n_sq = N_DVE + N_ACT
            rowsumsq = sp.tile([P, n_sq], mybir.dt.float32)

            # tile schedule: [fp8 gram xN_GRAM, bf16 xN_DVE] on Pool,
            # f32 xN_ACT on SP (interleaved from the start)
            kinds = (["g"] * N_GRAM + ["v"] * N_DVE)[:N_TILES - N_ACT]
            kinds += ["a"] * N_ACT
            # reorder: SP tiles issue independently; list order only
            # matters per engine. Keep program order = arrival order:
            order = []
            gi = 0
            for t in range(N_TILES):
                order.append(kinds[t])

            gram_seq = [i for i, k in enumerate(order) if k == "g"]
            cs_first = 0
            cs_last = N_TILES - 1

            sq_col = 0
            gram_done = 0
            for t, kind in enumerate(order):
                src = x_ext[t * TILE_ROWS:(t + 1) * TILE_ROWS, :]
                src = src.rearrange("(rb p) c -> p rb c", rb=RB, p=P)
                if kind == "g":
                    xt = io.tile([P, RB, COLS], mybir.dt.float8e4,
                                 tag="xg", name=f"xg{t}")
                    nc.gpsimd.dma_start(xt[:], src)
                    one_t = ones8
                elif kind == "v":
                    xt = io.tile([P, RB, COLS], mybir.dt.bfloat16,
                                 tag="xv", name=f"xv{t}", bufs=2)
                    nc.gpsimd.dma_start(xt[:], src)
                    one_t = ones_bf
                else:
                    xt = io.tile([P, RB, COLS], mybir.dt.float32,
                                 tag="xa", name=f"xa{t}", bufs=3)
                    nc.sync.dma_start(xt[:], src)
                    one_t = ones_f

                # column sums: cs[m, c] += sum_{p,rb} xt[p, rb, c*128+m]
                for c in range(N_CHUNKS):
                    for j in range(RB):
                        nc.tensor.matmul(
                            cs[:, c:c + 1],
                            lhsT=xt[:, j, c * CHUNK:(c + 1) * CHUNK],
                            rhs=one_t[:],
                            start=(t == cs_first and c == 0 and j == 0),
                            stop=(t == cs_last and c == N_CHUNKS - 1
                                  and j == RB - 1),
                        )

                if kind == "g":
                    # gram accumulate: gram[:, c*128:...] += X_c^T X_c
                    for c in range(N_CHUNKS):
                        for j in range(RB):
                            nc.tensor.matmul(
                                gram[:, c * CHUNK:(c + 1) * CHUNK],
                                lhsT=xt[:, j, c * CHUNK:(c + 1) * CHUNK],
                                rhs=xt[:, j, c * CHUNK:(c + 1) * CHUNK],
                                start=(gram_done == 0 and c == 0 and j == 0),
                                stop=(gram_done == N_GRAM - 1
                                      and c == N_CHUNKS - 1 and j == RB - 1),
                            )
                    gram_done += 1
                elif kind == "v":
                    scr_vb = io.tile([P, RB, COLS], mybir.dt.bfloat16,
                                     tag="scrv", name=f"scrv{t}", bufs=2)
                    nc.vector.tensor_mul(scr_vb[:], xt[:], xt[:])
                    nc.vector.tensor_scalar(
                        scr_vb[:], scr_vb[:], 1.0, None,
                        mybir.AluOpType.mult, mybir.AluOpType.add,
                        accum_out=rowsumsq[:, sq_col:sq_col + 1],
                    )
                    sq_col += 1
                else:
                    scr_a = io.tile([P, RB, COLS], mybir.dt.bfloat16,
                                    tag="scra", name=f"scra{t}", bufs=2)
                    nc.scalar.activation(
                        scr_a[:], xt[:],
                        mybir.ActivationFunctionType.Square,
                        accum_out=rowsumsq[:, sq_col:sq_col + 1],
                    )
                    sq_col += 1

            # ---- local tail ----
            # gram diagonal: gd = gram * eye (bf16), summed at 4x
            gd = sp.tile([P, N_CHUNKS, CHUNK], mybir.dt.bfloat16)
            nc.vector.tensor_mul(
                gd[:], gram[:].rearrange("p (c q) -> p c q", c=N_CHUNKS),
                eye4[:])
            sumsq_g = sp.tile([P, 1], mybir.dt.float32)
            nc.vector.tensor_scalar(
                gd[:], gd[:], 1.0, None,
                mybir.AluOpType.mult, mybir.AluOpType.add,
                accum_out=sumsq_g[:])
            # + DVE/ACT tile row sums
            sumsq_p = sp.tile([P, 1], mybir.dt.float32)
            nc.vector.tensor_reduce(
                out=sumsq_p[:], in_=rowsumsq[:],
                axis=mybir.AxisListType.X, op=mybir.AluOpType.add)
            nc.vector.tensor_add(sumsq_p[:], sumsq_p[:], sumsq_g[:])
            # colsum^2 per partition (ACT is idle by now)
            csq_scr = sp.tile([P, N_CHUNKS], mybir.dt.float32)
            csq = sp.tile([P, 1], mybir.dt.float32)
            nc.scalar.activation(
                csq_scr[:], cs[:], mybir.ActivationFunctionType.Square,
                accum_out=csq[:])
            # v[p] = (2/f)*csq - (2N/f)*sumsq
            a_sb = sp.tile([P, 1], mybir.dt.float32)
            nc.vector.tensor_scalar_mul(
                a_sb[:], sumsq_p[:], float(2.0 * N_CLASSES / FACTOR))
            v_sb = sp.tile([P, 1], mybir.dt.float32)
            nc.vector.scalar_tensor_tensor(
                out=v_sb[:], in0=csq[:], scalar=float(2.0 / FACTOR),
                in1=a_sb[:], op0=mybir.AluOpType.mult,
                op1=mybir.AluOpType.subtract)
            # collapse partitions: c_k = ones^T v  -> PSUM [1,1]
            ck_ps = ps.tile([1, 1], mybir.dt.float32)
            nc.tensor.matmul(ck_ps[:], lhsT=v_sb[:], rhs=ones_f[:],
                             start=True, stop=True)
            # place own scalar in recv column 0 (partition 0)
            nc.vector.tensor_copy(bass.AP(recv, 0, [[8, 1], [1, 1]]),
                                  ck_ps[:])

    # ---- cross-core sum: single-shot all-to-all of the 8 scalars ----
    # Two waves of 7 single-slot broadcasts (slot d -> core ^ d). Wave 2
    # is a flush: its remote-sem updates trail wave 1's data by a full
    # wire round trip per DMA-engine FIFO, closing the window where a
    # remote sem becomes visible before the SBUF bytes do. Batches of 7
    # stay within the SWDGE broadcast-group FIFO.
    with nc.Block("a2a"):
        def bcol(d):
            return bass.AP(recvb, d, [[8, P], [1, 1]])
        for wave, tgt in enumerate((rcol, bcol)):
            for d in range(1, 8):
                rd = [None] * 8
                rd[d] = (0, d)
                nc.gpsimd.remote_dma_broadcast(
                    tgt(d), rcol(0), remote_sem=dsem, local_sem=lsem,
                    rdests=rd).then_inc(prep_sem, 1)
            nc.gpsimd.wait_ge(prep_sem, 7 * (wave + 1))
            nc.gpsimd.trigger_dma(7)
        nc.vector.wait_ge(dsem, 28)
        # settle margin on top of the flush round trip
        nc.vector.tensor_copy(bass.AP(pad, 0, [[512, P], [1, 512]]),
                              bass.AP(pad, 0, [[512, P], [1, 512]]))
        nc.vector.tensor_add(bass.AP(radd, 0, [[4, P], [1, 4]]),
                             bass.AP(recv, 0, [[8, P], [1, 4]]),
                             bass.AP(recv, 4, [[8, P], [1, 4]])
                             ).then_inc(cp_sem, 1)
        nc.vector.wait_ge(cp_sem, 1)
        nc.vector.tensor_add(bass.AP(radd, 0, [[4, P], [1, 2]]),
                             bass.AP(radd, 0, [[4, P], [1, 2]]),
                             bass.AP(radd, 2, [[4, P], [1, 2]])
                             ).then_inc(cp_sem, 1)
        nc.vector.wait_ge(cp_sem, 2)
        nc.vector.tensor_add(bass.AP(tot, 0, [[1, P], [1, 1]]),
                             bass.AP(radd, 0, [[4, P], [1, 1]]),
                             bass.AP(radd, 1, [[4, P], [1, 1]])
                             ).then_inc(cp_sem, 1)
        nc.sync.wait_ge(cp_sem, 3)
        nc.sync.dma_start(out_ext[:], bass.AP(tot, 0, [[1, 1], [1, 1]])
                          ).then_inc(out_sem, 16)
        nc.sync.wait_ge(out_sem, 16)

    nc.finalize()
    es.close()
    return nc


_NC_CACHE = None


def _get_nc():
    global _NC_CACHE
    if _NC_CACHE is None:
        _NC_CACHE = _build()
    return _NC_CACHE


def _run(anchors: np.ndarray, trace: bool = False):
    """Returns (loss_scalar, BassKernelResults)."""
    x = np.asarray(anchors, dtype=np.float32).reshape(N_CLASSES, D)
    in_maps = [
        {"anchors": np.ascontiguousarray(x[:, i * COLS:(i + 1) * COLS])}
        for i in range(N_CORES)
    ]
    nc = _get_nc()
    res = run_bass_kernel_spmd(nc, in_maps, core_ids=list(range(N_CORES)),
                               trace=trace)
    loss = np.float32(np.asarray(res.results[0]["out"]).reshape(())[()])
    return loss, res


def kernel(anchors: np.ndarray) -> np.ndarray:
    loss, _ = _run(anchors)
    return np.asarray(loss, dtype=np.float32).reshape(())


# revision 9
# speedup vs baseline: 2.2654x; 1.0667x over previous
"""AnchorLoss distributed Trainium2 kernel (8 NeuronCores).

reference math (anchors: [8192, 8, 512] f32):
    x = anchors.reshape(8192, 4096)
    loss = -(2*N*sum(x*x) - 2*sum(colsum(x)^2)) / sqrt(512)

Strategy: shard COLUMNS across the 8 cores (512 columns each); every
cross-core quantity is then a single scalar per core. Each core streams
its [8192, 512] f32 slice as 16 row-tiles of [128, 4, 512]:

  - 11 "gram" tiles are cast f32->fp8e4 inside the Pool SWDGE DMA (HBM
    still reads every f32 byte once); PE accumulates per-chunk Gram
    matrices X_c^T X_c into one PSUM bank [128, 4x128] whose diagonal
    is the tile's sum of squares. PE also column-sums every tile via
    ones-vector matmuls into a second PSUM bank [128, 4].
  - 2 tiles are cast f32->bf16 (Pool DMA) and squared on DVE
    (2x tensor_mul + 4x tensor_scalar accumulate).
  - 3 tiles stay f32 on the SP HWDGE ring and are squared on ScalarE.

This splits the elementwise-square roofline across PE/DVE/ACT while
Pool+SP share the DMA bytes, so all five engines run ~balanced.

The per-core partial  c_k = (2/f)*||colsum_k||^2 - (2N/f)*sumsq_k  is
collapsed to a scalar with a ones^T matmul, then summed across cores
WITHOUT the 15us collective: a raw post-tile block runs a single-shot
all-to-all of the 8 scalars via XOR-relative remote_dma_broadcast
(7 single-slot broadcasts, slot d targets core ^ d; the hardware XORs
physical ids, which relabels peers but stays a bijection, so the sum
is invariant). Each core then tree-adds the 8 values and SP DMAs the
total to "out"; the host reads core 0.
"""

import numpy as np
from contextlib import ExitStack

from concourse import bacc, bass, tile, mybir
from concourse.bass_utils import run_bass_kernel_spmd

# The axon client container has no /dev/neuron*, so the driver ioctls
# behind these routing lookups fail. The simulator only needs a sane
# single-device identity mapping (8 cores on device 0); the real NEFF
# resolves XOR-relative routing on-device and never reads these.
import concourse.libnrt as _lnrt
import concourse.bass_interp as _bi
try:
    _lnrt.get_trn2_nc_mapping()
except Exception:
    _IDENT = {(0, i): i for i in range(8)}
    _RID = {0: 0}
    _lnrt.get_trn2_nc_mapping = lambda: _IDENT
    _lnrt.get_device_id_to_routing_id_mapping = lambda: _RID
    _bi.get_device_id_to_routing_id_mapping = lambda: _RID

N_CORES = 8
N_CLASSES = 8192
D = 4096                          # 8 * 512 flattened embedding dim
COLS = D // N_CORES               # 512 columns per core
P = 128                           # partitions
RB = 4                            # row-blocks per tile
TILE_ROWS = P * RB                # 512 rows per tile
N_TILES = N_CLASSES // TILE_ROWS  # 16
CHUNK = 128
N_CHUNKS = COLS // CHUNK          # 4
FACTOR = float(np.sqrt(np.float32(512.0)))

N_GRAM = 11                       # fp8 tiles -> PE gram diag
N_DVE = 2                         # bf16 tiles -> DVE squares
N_ACT = N_TILES - N_GRAM - N_DVE  # f32 tiles (SP DMA) -> ACT squares
PE_WARMUP = 20                    # dummy matmuls to ramp the PE p-state


def _build():
    nc = bacc.Bacc(None, num_devices=N_CORES)
    x_ext = nc.declare_dram_parameter(
        "anchors", [N_CLASSES, COLS], mybir.dt.float32, isOutput=False
    )
    out_ext = nc.declare_dram_parameter(
        "out", [1, 1], mybir.dt.float32, isOutput=True
    )

    es = ExitStack()
    # raw SBUF tensors shared with the post-tile all-to-all block
    recv = es.enter_context(nc.sbuf_tensor("recv8", [P, 8], mybir.dt.float32))
    recvb = es.enter_context(nc.sbuf_tensor("recvb8", [P, 8], mybir.dt.float32))
    radd = es.enter_context(nc.sbuf_tensor("radd", [P, 4], mybir.dt.float32))
    tot = es.enter_context(nc.sbuf_tensor("tot", [P, 1], mybir.dt.float32))
    pad = es.enter_context(nc.sbuf_tensor("pad", [P, 512], mybir.dt.float32))
    lsem = nc.alloc_semaphore("aa_lsem")
    prep_sem = nc.alloc_semaphore("aa_prep")
    dsem = nc.alloc_semaphore("aa_dsem")
    cp_sem = nc.alloc_semaphore("aa_cp")
    out_sem = nc.alloc_semaphore("aa_out")

    def rcol(d):
        return bass.AP(recv, d, [[8, P], [1, 1]])

    with tile.TileContext(nc) as tc:
        with (
            tc.tile_pool(name="io", bufs=6) as io,
            tc.tile_pool(name="small", bufs=1) as sp,
            tc.tile_pool(name="psum", bufs=1, space="PSUM") as ps,
        ):
            # constants (keep Pool free: build on DVE where possible)
            ones8 = sp.tile([P, 1], mybir.dt.float8e4)
            nc.vector.memset(ones8[:], 1.0)
            ones_bf = sp.tile([P, 1], mybir.dt.bfloat16)
            nc.vector.memset(ones_bf[:], 1.0)
            ones_f = sp.tile([P, 1], mybir.dt.float32)
            nc.vector.memset(ones_f[:], 1.0)
            wones = sp.tile([P, CHUNK], mybir.dt.bfloat16)
            nc.vector.memset(wones[:], 0.001)
            nc.vector.memset(bass.AP(recv, 0, [[8, P], [1, 8]]), 0.0)
            nc.vector.memset(bass.AP(pad, 0, [[512, P], [1, 512]]), 0.0)
            # identity mask for the gram diagonal: eye[p, q] = (q == p)
            iq = sp.tile([P, CHUNK], mybir.dt.float32)
            nc.gpsimd.iota(iq[:], [[1, CHUNK]], channel_multiplier=0,
                           allow_small_or_imprecise_dtypes=True)
            ip = sp.tile([P, 1], mybir.dt.float32)
            nc.gpsimd.iota(ip[:], [[0, 1]], channel_multiplier=1,
                           allow_small_or_imprecise_dtypes=True)
            eye = sp.tile([P, CHUNK], mybir.dt.float32)
            nc.vector.tensor_tensor(
                eye[:], iq[:], ip[:].broadcast_to([P, CHUNK]),
                mybir.AluOpType.is_equal)


            # ACT table preload: tiny square so LoadActFuncSet runs early
            warm_a = sp.tile([P, 1], mybir.dt.float32)
            nc.scalar.activation(warm_a[:], ones_f[:],
                                 mybir.ActivationFunctionType.Square)

            # PE p-state warmup: dummy matmuls while DMAs stream
            warm_ps = ps.tile([P, CHUNK], mybir.dt.float32)
            for i in range(PE_WARMUP):
                nc.tensor.matmul(warm_ps[:], lhsT=wones[:], rhs=wones[:],
                                 start=True, stop=True)

            # PSUM accumulators. All 4 column-chunks of every gram tile
            # accumulate into ONE [128,128] bank: its diagonal is then
            # sum_c ||col_{c,q}||^2, i.e. exactly the per-q partial sums
            # of squares (the off-diagonal cross terms are never read).
            gram = ps.tile([P, CHUNK], mybir.dt.float32)
            cs = ps.tile([P, N_CHUNKS], mybir.dt.float32)

            # accumulator columns for DVE/ACT pieces' row-sums of squares
            rowsumsq = sp.tile([P, 8], mybir.dt.float32)
            nc.vector.memset(rowsumsq[:], 0.0)

            # Tile pieces in per-queue issue order. Pool streams gram
            # tiles fp8 with one bf16 (DVE-squared) tile mid-stream and
            # the last bf16 tile as two tail halves; SP streams 3 f32
            # tiles for ACT. Pieces: (kind, tile_idx, rb_lo, rb_hi).
            pool_q = []
            sp_q = []
            g_ids = list(range(N_GRAM))                  # tiles 0..10
            v1, v2 = N_GRAM, N_GRAM + 1                  # tiles 11, 12
            a_ids = [N_GRAM + 2, N_GRAM + 3, N_GRAM + 4]  # tiles 13..15
            for i, g in enumerate(g_ids):
                if i == 6:
                    pool_q.append(("v", v1, 0, RB))
                pool_q.append(("g", g, 0, RB))
            pool_q.append(("v", v2, 0, 2))
            pool_q.append(("v", v2, 2, RB))
            for a in a_ids:
                sp_q.append(("a", a, 0, RB))

            # arrival-time estimate to order the consumer-side program
            POOL_D, SP_D = 1883.0, 1717.0
            BYTE_NS = 0.3855

            def piece_bytes(kind, nrb):
                per = {"g": 1, "v": 2, "a": 4}[kind]
                return nrb * COLS * per

            merged = []
            t = 100.0
            for pc in pool_q:
                t += piece_bytes(pc[0], pc[3] - pc[2]) * BYTE_NS
                merged.append((t + POOL_D, pc))
            t = 100.0
            for pc in sp_q:
                t += piece_bytes(pc[0], pc[3] - pc[2]) * BYTE_NS
                merged.append((t + SP_D, pc))
            merged.sort(key=lambda m: m[0])

            n_gram_mm = sum(N_CHUNKS * (pc[3] - pc[2])
                            for _, pc in merged if pc[0] == "g")
            n_cs_mm = sum(N_CHUNKS * (pc[3] - pc[2]) for _, pc in merged)

            tiles = {}   # tile_idx -> sbuf tile (DMA'd whole per piece set)
            sq_col = 0
            gram_mm = 0
            cs_mm = 0
            for _, (kind, ti, rb_lo, rb_hi) in merged:
                nrb = rb_hi - rb_lo
                src = x_ext[ti * TILE_ROWS + rb_lo * P:
                            ti * TILE_ROWS + rb_hi * P, :]
                src = src.rearrange("(rb p) c -> p rb c", rb=nrb, p=P)
                if kind == "g":
                    xt = io.tile([P, nrb, COLS], mybir.dt.float8e4,
                                 tag="xg", name=f"xg{ti}_{rb_lo}")
                    nc.gpsimd.dma_start(xt[:], src)
                    one_t = ones8
                elif kind == "v":
                    xt = io.tile([P, nrb, COLS], mybir.dt.bfloat16,
                                 tag=f"xv{nrb}", name=f"xv{ti}_{rb_lo}",
                                 bufs=2)
                    nc.gpsimd.dma_start(xt[:], src)
                    one_t = ones_bf
                else:
                    xt = io.tile([P, nrb, COLS], mybir.dt.float32,
                                 tag="xa", name=f"xa{ti}_{rb_lo}", bufs=2)
                    nc.sync.dma_start(xt[:], src)
                    one_t = ones_f

                # column sums: cs[m, c] += sum_{p,rb} xt[p, rb, c*128+m]
                for c in range(N_CHUNKS):
                    for j in range(nrb):
                        cs_mm += 1
                        nc.tensor.matmul(
                            cs[:, c:c + 1],
                            lhsT=xt[:, j, c * CHUNK:(c + 1) * CHUNK],
                            rhs=one_t[:],
                            start=(cs_mm == 1), stop=(cs_mm == n_cs_mm),
                        )

                if kind == "g":
                    for c in range(N_CHUNKS):
                        for j in range(nrb):
                            gram_mm += 1
                            nc.tensor.matmul(
                                gram[:],
                                lhsT=xt[:, j, c * CHUNK:(c + 1) * CHUNK],
                                rhs=xt[:, j, c * CHUNK:(c + 1) * CHUNK],
                                start=(gram_mm == 1),
                                stop=(gram_mm == n_gram_mm),
                            )
                elif kind == "v":
                    scr_vb = io.tile([P, nrb, COLS], mybir.dt.bfloat16,
                                     tag=f"scrv{nrb}", name=f"sv{ti}_{rb_lo}",
                                     bufs=2)
                    nc.vector.tensor_mul(scr_vb[:], xt[:], xt[:])
                    nc.vector.tensor_scalar(
                        scr_vb[:], scr_vb[:], 1.0, None,
                        mybir.AluOpType.mult, mybir.AluOpType.add,
                        accum_out=rowsumsq[:, sq_col:sq_col + 1],
                    )
                    sq_col += 1
                else:
                    scr_a = io.tile([P, nrb, COLS], mybir.dt.bfloat16,
                                    tag="scra", name=f"sa{ti}_{rb_lo}",
                                    bufs=2)
                    nc.scalar.activation(
                        scr_a[:], xt[:],
                        mybir.ActivationFunctionType.Square,
                        accum_out=rowsumsq[:, sq_col:sq_col + 1],
                    )
                    sq_col += 1
            assert sq_col <= 8

            # ---- local tail ----
            # gram diagonal -> per-partition gram sum of squares
            gd = sp.tile([P, CHUNK], mybir.dt.float32)
            nc.vector.tensor_mul(gd[:], gram[:], eye[:])
            sumsq_g = sp.tile([P, 1], mybir.dt.float32)
            nc.vector.tensor_scalar(
                gd[:], gd[:], 1.0, None,
                mybir.AluOpType.mult, mybir.AluOpType.add,
                accum_out=sumsq_g[:])
            # + DVE/ACT tile row sums
            sumsq_p = sp.tile([P, 1], mybir.dt.float32)
            nc.vector.tensor_reduce(
                out=sumsq_p[:], in_=rowsumsq[:],
                axis=mybir.AxisListType.X, op=mybir.AluOpType.add)
            nc.vector.tensor_add(sumsq_p[:], sumsq_p[:], sumsq_g[:])
            # colsum^2 per partition (ACT is idle by now)
            csq_scr = sp.tile([P, N_CHUNKS], mybir.dt.float32)
            csq = sp.tile([P, 1], mybir.dt.float32)
            nc.scalar.activation(
                csq_scr[:], cs[:], mybir.ActivationFunctionType.Square,
                accum_out=csq[:])
            # v[p] = (2/f)*csq - (2N/f)*sumsq
            a_sb = sp.tile([P, 1], mybir.dt.float32)
            nc.vector.tensor_scalar_mul(
                a_sb[:], sumsq_p[:], float(2.0 * N_CLASSES / FACTOR))
            v_sb = sp.tile([P, 1], mybir.dt.float32)
            nc.vector.scalar_tensor_tensor(
                out=v_sb[:], in0=csq[:], scalar=float(2.0 / FACTOR),
                in1=a_sb[:], op0=mybir.AluOpType.mult,
                op1=mybir.AluOpType.subtract)
            # collapse partitions: c_k = ones^T v  -> PSUM [1,1]
            ck_ps = ps.tile([1, 1], mybir.dt.float32)
            nc.tensor.matmul(ck_ps[:], lhsT=v_sb[:], rhs=ones_f[:],
                             start=True, stop=True)
            # place own scalar in recv column 0 (partition 0)
            nc.vector.tensor_copy(bass.AP(recv, 0, [[8, 1], [1, 1]]),
                                  ck_ps[:])

    # ---- cross-core sum: single-shot all-to-all of the 8 scalars ----
    # Two waves of 7 single-slot broadcasts (slot d -> core ^ d). Wave 2
    # is a flush: its remote-sem updates trail wave 1's data by a full
    # wire round trip per DMA-engine FIFO, closing the window where a
    # remote sem becomes visible before the SBUF bytes do. Batches of 7
    # stay within the SWDGE broadcast-group FIFO.
    with nc.Block("a2a"):
        def bcol(d):
            return bass.AP(recvb, d, [[8, P], [1, 1]])
        for wave, tgt in enumerate((rcol, bcol)):
            for d in range(1, 8):
                rd = [None] * 8
                rd[d] = (0, d)
                nc.gpsimd.remote_dma_broadcast(
                    tgt(d), rcol(0), remote_sem=dsem, local_sem=lsem,
                    rdests=rd).then_inc(prep_sem, 1)
            nc.gpsimd.wait_ge(prep_sem, 7 * (wave + 1))
            nc.gpsimd.trigger_dma(7)
        nc.vector.wait_ge(dsem, 28)
        # settle margin on top of the flush round trip
        nc.vector.tensor_copy(bass.AP(pad, 0, [[512, P], [1, 512]]),
                              bass.AP(pad, 0, [[512, P], [1, 512]]))
        nc.vector.tensor_add(bass.AP(radd, 0, [[4, P], [1, 4]]),
                             bass.AP(recv, 0, [[8, P], [1, 4]]),
                             bass.AP(recv, 4, [[8, P], [1, 4]])
                             ).then_inc(cp_sem, 1)
        nc.vector.wait_ge(cp_sem, 1)
        nc.vector.tensor_add(bass.AP(radd, 0, [[4, P], [1, 2]]),
                             bass.AP(radd, 0, [[4, P], [1, 2]]),
                             bass.AP(radd, 2, [[4, P], [1, 2]])
                             ).then_inc(cp_sem, 1)
        nc.vector.wait_ge(cp_sem, 2)
        nc.vector.tensor_add(bass.AP(tot, 0, [[1, P], [1, 1]]),
                             bass.AP(radd, 0, [[4, P], [1, 1]]),
                             bass.AP(radd, 1, [[4, P], [1, 1]])
                             ).then_inc(cp_sem, 1)
        nc.sync.wait_ge(cp_sem, 3)
        nc.sync.dma_start(out_ext[:], bass.AP(tot, 0, [[1, 1], [1, 1]])
                          ).then_inc(out_sem, 16)
        nc.sync.wait_ge(out_sem, 16)

    nc.finalize()
    es.close()
    return nc


_NC_CACHE = None


def _get_nc():
    global _NC_CACHE
    if _NC_CACHE is None:
        _NC_CACHE = _build()
    return _NC_CACHE


def _run(anchors: np.ndarray, trace: bool = False):
    """Returns (loss_scalar, BassKernelResults)."""
    x = np.asarray(anchors, dtype=np.float32).reshape(N_CLASSES, D)
    in_maps = [
        {"anchors": np.ascontiguousarray(x[:, i * COLS:(i + 1) * COLS])}
        for i in range(N_CORES)
    ]
    nc = _get_nc()
    res = run_bass_kernel_spmd(nc, in_maps, core_ids=list(range(N_CORES)),
                               trace=trace)
    loss = np.float32(np.asarray(res.results[0]["out"]).reshape(())[()])
    return loss, res


def kernel(anchors: np.ndarray) -> np.ndarray:
    loss, _ = _run(anchors)
    return np.asarray(loss, dtype=np.float32).reshape(())


# revision 16
# speedup vs baseline: 2.3403x; 1.0331x over previous
"""AnchorLoss distributed Trainium2 kernel (8 NeuronCores).

reference math (anchors: [8192, 8, 512] f32):
    x = anchors.reshape(8192, 4096)
    loss = -(2*N*sum(x*x) - 2*sum(colsum(x)^2)) / sqrt(512)

Strategy: shard COLUMNS across the 8 cores (512 columns each); every
cross-core quantity is then a single scalar per core. Each core streams
its [8192, 512] f32 slice as 16 row-tiles of [128, 4, 512]:

  - 11 "gram" tiles are cast f32->fp8e4 inside the Pool SWDGE DMA (HBM
    still reads every f32 byte once); PE accumulates per-chunk Gram
    matrices X_c^T X_c into one PSUM bank [128, 4x128] whose diagonal
    is the tile's sum of squares. PE also column-sums every tile via
    ones-vector matmuls into a second PSUM bank [128, 4].
  - 2 tiles are cast f32->bf16 (Pool DMA) and squared on DVE
    (2x tensor_mul + 4x tensor_scalar accumulate).
  - 3 tiles stay f32 on the SP HWDGE ring and are squared on ScalarE.

This splits the elementwise-square roofline across PE/DVE/ACT while
Pool+SP share the DMA bytes, so all five engines run ~balanced.

The per-core partial  c_k = (2/f)*||colsum_k||^2 - (2N/f)*sumsq_k  is
collapsed to a scalar with a ones^T matmul, then summed across cores
WITHOUT the 15us collective: a raw post-tile block runs a single-shot
all-to-all of the 8 scalars via XOR-relative remote_dma_broadcast
(7 single-slot broadcasts, slot d targets core ^ d; the hardware XORs
physical ids, which relabels peers but stays a bijection, so the sum
is invariant). Each core then tree-adds the 8 values and SP DMAs the
total to "out"; the host reads core 0.
"""

import numpy as np
from contextlib import ExitStack

from concourse import bacc, bass, tile, mybir
from concourse.bass_utils import run_bass_kernel_spmd

# The axon client container has no /dev/neuron*, so the driver ioctls
# behind these routing lookups fail. The simulator only needs a sane
# single-device identity mapping (8 cores on device 0); the real NEFF
# resolves XOR-relative routing on-device and never reads these.
import concourse.libnrt as _lnrt
import concourse.bass_interp as _bi
try:
    _lnrt.get_trn2_nc_mapping()
except Exception:
    _IDENT = {(0, i): i for i in range(8)}
    _RID = {0: 0}
    _lnrt.get_trn2_nc_mapping = lambda: _IDENT
    _lnrt.get_device_id_to_routing_id_mapping = lambda: _RID
    _bi.get_device_id_to_routing_id_mapping = lambda: _RID

N_CORES = 8
N_CLASSES = 8192
D = 4096                          # 8 * 512 flattened embedding dim
COLS = D // N_CORES               # 512 columns per core
P = 128                           # partitions
RB = 4                            # row-blocks per tile
TILE_ROWS = P * RB                # 512 rows per tile
N_TILES = N_CLASSES // TILE_ROWS  # 16
CHUNK = 128
N_CHUNKS = COLS // CHUNK          # 4
FACTOR = float(np.sqrt(np.float32(512.0)))

N_GRAM = 11                       # fp8 tiles -> PE gram diag
N_DVE = 2                         # bf16 tiles -> DVE squares
N_ACT = N_TILES - N_GRAM - N_DVE  # f32 tiles (SP DMA) -> ACT squares
PE_WARMUP = 20                    # dummy matmuls to ramp the PE p-state


def _build():
    nc = bacc.Bacc(None, num_devices=N_CORES)
    x_ext = nc.declare_dram_parameter(
        "anchors", [N_CLASSES, COLS], mybir.dt.float32, isOutput=False
    )
    out_ext = nc.declare_dram_parameter(
        "out", [1, 1], mybir.dt.float32, isOutput=True
    )

    es = ExitStack()
    # raw SBUF tensors shared with the post-tile all-to-all block
    recv = es.enter_context(nc.sbuf_tensor("recv8", [P, 8], mybir.dt.float32))
    recvb = es.enter_context(nc.sbuf_tensor("recvb8", [P, 8], mybir.dt.float32))
    radd = es.enter_context(nc.sbuf_tensor("radd", [P, 4], mybir.dt.float32))
    tot = es.enter_context(nc.sbuf_tensor("tot", [P, 1], mybir.dt.float32))
    pad = es.enter_context(nc.sbuf_tensor("pad", [P, 512], mybir.dt.float32))
    lsem = nc.alloc_semaphore("aa_lsem")
    prep_sem = nc.alloc_semaphore("aa_prep")
    dsem = nc.alloc_semaphore("aa_dsem")
    cp_sem = nc.alloc_semaphore("aa_cp")
    out_sem = nc.alloc_semaphore("aa_out")

    def rcol(d):
        return bass.AP(recv, d, [[8, P], [1, 1]])

    with tile.TileContext(nc) as tc:
        with (
            tc.tile_pool(name="io", bufs=6) as io,
            tc.tile_pool(name="small", bufs=1) as sp,
            tc.tile_pool(name="psum", bufs=1, space="PSUM") as ps,
        ):
            # constants (keep Pool free: build on DVE where possible)
            ones8 = sp.tile([P, 1], mybir.dt.float8e4)
            nc.vector.memset(ones8[:], 1.0)
            ones_bf = sp.tile([P, 1], mybir.dt.bfloat16)
            nc.vector.memset(ones_bf[:], 1.0)
            ones_f = sp.tile([P, 1], mybir.dt.float32)
            nc.vector.memset(ones_f[:], 1.0)
            wones = sp.tile([P, CHUNK], mybir.dt.bfloat16)
            nc.vector.memset(wones[:], 0.001)
            nc.vector.memset(bass.AP(recv, 0, [[8, P], [1, 8]]), 0.0)
            nc.vector.memset(bass.AP(pad, 0, [[512, P], [1, 512]]), 0.0)
            # identity mask for the gram diagonal: eye[p, q] = (q == p)
            iq = sp.tile([P, CHUNK], mybir.dt.float32)
            nc.gpsimd.iota(iq[:], [[1, CHUNK]], channel_multiplier=0,
                           allow_small_or_imprecise_dtypes=True)
            ip = sp.tile([P, 1], mybir.dt.float32)
            nc.gpsimd.iota(ip[:], [[0, 1]], channel_multiplier=1,
                           allow_small_or_imprecise_dtypes=True)
            eye = sp.tile([P, CHUNK], mybir.dt.float32)
            nc.vector.tensor_tensor(
                eye[:], iq[:], ip[:].broadcast_to([P, CHUNK]),
                mybir.AluOpType.is_equal)


            # ACT table preload: tiny square so LoadActFuncSet runs early
            warm_a = sp.tile([P, 1], mybir.dt.float32)
            nc.scalar.activation(warm_a[:], ones_f[:],
                                 mybir.ActivationFunctionType.Square)

            # PE p-state warmup: dummy matmuls while DMAs stream
            warm_ps = ps.tile([P, CHUNK], mybir.dt.float32)
            for i in range(PE_WARMUP):
                nc.tensor.matmul(warm_ps[:], lhsT=wones[:], rhs=wones[:],
                                 start=True, stop=True)

            # PSUM accumulators. All 4 column-chunks of every gram tile
            # accumulate into ONE [128,128] bank: its diagonal is then
            # sum_c ||col_{c,q}||^2, i.e. exactly the per-q partial sums
            # of squares (the off-diagonal cross terms are never read).
            gramA = ps.tile([P, CHUNK], mybir.dt.float32, name="gramA")
            gramB = ps.tile([P, CHUNK], mybir.dt.float32, name="gramB")
            cs = ps.tile([P, N_CHUNKS], mybir.dt.float32)

            # accumulator columns for DVE/ACT pieces' row-sums of squares
            rowsumsq = sp.tile([P, 8], mybir.dt.float32)
            nc.vector.memset(rowsumsq[:], 0.0)

            # Tile pieces in per-queue issue order. Pool streams gram
            # tiles fp8 with one bf16 (DVE-squared) tile mid-stream and
            # the last bf16 tile as two tail halves; SP streams 3 f32
            # tiles for ACT. Pieces: (kind, tile_idx, rb_lo, rb_hi).
            # Pool order: 5 gram tiles, both bf16 tiles mid-stream (DVE
            # squares them while streaming), then the remaining gram
            # tiles with the last one as two halves (cheap tail: a half
            # gram is ~0.45us of PE work). Gram bank A covers the first
            # 9 gram tiles and stops early so its diagonal extraction
            # overlaps the stream; bank B covers the last ~2.
            pool_q = []
            sp_q = []
            g_ids = list(range(N_GRAM))                  # tiles 0..10
            v1, v2 = N_GRAM, N_GRAM + 1                  # tiles 11, 12
            a_ids = [N_GRAM + 2, N_GRAM + 3, N_GRAM + 4]  # tiles 13..15
            for i, g in enumerate(g_ids[:5]):
                pool_q.append(("g", g, 0, RB))
            pool_q.append(("v", v1, 0, RB))
            pool_q.append(("v", v2, 0, RB))
            for g in g_ids[5:-1]:
                pool_q.append(("g", g, 0, RB))
            pool_q.append(("g", g_ids[-1], 0, 2))
            pool_q.append(("g", g_ids[-1], 2, RB))
            for a in a_ids:
                sp_q.append(("a", a, 0, RB))
            N_BANK_A = 9                                 # gram tiles in bank A

            # arrival-time estimate to order the consumer-side program
            POOL_D, SP_D = 1883.0, 1717.0
            BYTE_NS = 0.3855

            def piece_bytes(kind, nrb):
                per = {"g": 1, "v": 2, "a": 4}[kind]
                return nrb * COLS * per

            merged = []
            t = 100.0
            for pc in pool_q:
                t += piece_bytes(pc[0], pc[3] - pc[2]) * BYTE_NS
                merged.append((t + POOL_D, pc))
            t = 100.0
            for pc in sp_q:
                t += piece_bytes(pc[0], pc[3] - pc[2]) * BYTE_NS
                merged.append((t + SP_D, pc))
            merged.sort(key=lambda m: m[0])

            bank_a_tiles = set(g_ids[:N_BANK_A])
            n_gram_mm_a = sum(N_CHUNKS * (pc[3] - pc[2]) for _, pc in merged
                              if pc[0] == "g" and pc[1] in bank_a_tiles)
            n_gram_mm_b = sum(N_CHUNKS * (pc[3] - pc[2]) for _, pc in merged
                              if pc[0] == "g" and pc[1] not in bank_a_tiles)
            n_cs_mm = sum(N_CHUNKS * (pc[3] - pc[2]) for _, pc in merged)

            sq_col = 0
            gram_mm_a = 0
            gram_mm_b = 0
            cs_mm = 0
            for _, (kind, ti, rb_lo, rb_hi) in merged:
                nrb = rb_hi - rb_lo
                src = x_ext[ti * TILE_ROWS + rb_lo * P:
                            ti * TILE_ROWS + rb_hi * P, :]
                src = src.rearrange("(rb p) c -> p rb c", rb=nrb, p=P)
                if kind == "g":
                    xt = io.tile([P, nrb, COLS], mybir.dt.float8e4,
                                 tag="xg", name=f"xg{ti}_{rb_lo}")
                    nc.gpsimd.dma_start(xt[:], src)
                    one_t = ones8
                elif kind == "v":
                    xt = io.tile([P, nrb, COLS], mybir.dt.bfloat16,
                                 tag=f"xv{nrb}", name=f"xv{ti}_{rb_lo}",
                                 bufs=2)
                    nc.gpsimd.dma_start(xt[:], src)
                    one_t = ones_bf
                else:
                    xt = io.tile([P, nrb, COLS], mybir.dt.float32,
                                 tag="xa", name=f"xa{ti}_{rb_lo}", bufs=2)
                    nc.sync.dma_start(xt[:], src)
                    one_t = ones_f

                # column sums: cs[m, c] += sum_{p,rb} xt[p, rb, c*128+m]
                for c in range(N_CHUNKS):
                    for j in range(nrb):
                        cs_mm += 1
                        nc.tensor.matmul(
                            cs[:, c:c + 1],
                            lhsT=xt[:, j, c * CHUNK:(c + 1) * CHUNK],
                            rhs=one_t[:],
                            start=(cs_mm == 1), stop=(cs_mm == n_cs_mm),
                        )

                if kind == "g":
                    in_a = ti in bank_a_tiles
                    bank = gramA if in_a else gramB
                    for c in range(N_CHUNKS):
                        for j in range(nrb):
                            if in_a:
                                gram_mm_a += 1
                                st = gram_mm_a == 1
                                sp_ = gram_mm_a == n_gram_mm_a
                            else:
                                gram_mm_b += 1
                                st = gram_mm_b == 1
                                sp_ = gram_mm_b == n_gram_mm_b
                            nc.tensor.matmul(
                                bank[:],
                                lhsT=xt[:, j, c * CHUNK:(c + 1) * CHUNK],
                                rhs=xt[:, j, c * CHUNK:(c + 1) * CHUNK],
                                start=st, stop=sp_,
                            )
                elif kind == "v":
                    scr_vb = io.tile([P, nrb, COLS], mybir.dt.bfloat16,
                                     tag=f"scrv{nrb}", name=f"sv{ti}_{rb_lo}",
                                     bufs=2)
                    nc.vector.tensor_mul(scr_vb[:], xt[:], xt[:])
                    nc.vector.tensor_scalar(
                        scr_vb[:], scr_vb[:], 1.0, None,
                        mybir.AluOpType.mult, mybir.AluOpType.add,
                        accum_out=rowsumsq[:, sq_col:sq_col + 1],
                    )
                    sq_col += 1
                else:
                    scr_a = io.tile([P, nrb, COLS], mybir.dt.bfloat16,
                                    tag="scra", name=f"sa{ti}_{rb_lo}",
                                    bufs=2)
                    nc.scalar.activation(
                        scr_a[:], xt[:],
                        mybir.ActivationFunctionType.Square,
                        accum_out=rowsumsq[:, sq_col:sq_col + 1],
                    )
                    sq_col += 1
            assert sq_col <= 8

            # ---- local tail ----
            # gram diagonals -> per-partition gram sums of squares.
            # Bank A closes mid-stream, so its extraction overlaps the
            # remaining DMAs; only bank B's extraction trails the stream.
            gdA = sp.tile([P, CHUNK], mybir.dt.float32)
            nc.vector.tensor_mul(gdA[:], gramA[:], eye[:])
            sumsq_ga = sp.tile([P, 1], mybir.dt.float32)
            nc.vector.tensor_scalar(
                gdA[:], gdA[:], 1.0, None,
                mybir.AluOpType.mult, mybir.AluOpType.add,
                accum_out=sumsq_ga[:])
            gdB = sp.tile([P, CHUNK], mybir.dt.float32)
            nc.vector.tensor_mul(gdB[:], gramB[:], eye[:])
            sumsq_gb = sp.tile([P, 1], mybir.dt.float32)
            nc.vector.tensor_scalar(
                gdB[:], gdB[:], 1.0, None,
                mybir.AluOpType.mult, mybir.AluOpType.add,
                accum_out=sumsq_gb[:])
            # + DVE/ACT tile row sums
            sumsq_p = sp.tile([P, 1], mybir.dt.float32)
            nc.vector.tensor_reduce(
                out=sumsq_p[:], in_=rowsumsq[:],
                axis=mybir.AxisListType.X, op=mybir.AluOpType.add)
            nc.vector.tensor_add(sumsq_p[:], sumsq_p[:], sumsq_ga[:])
            nc.vector.tensor_add(sumsq_p[:], sumsq_p[:], sumsq_gb[:])
            # colsum^2 per partition (ACT is idle by now)
            csq_scr = sp.tile([P, N_CHUNKS], mybir.dt.float32)
            csq = sp.tile([P, 1], mybir.dt.float32)
            nc.scalar.activation(
                csq_scr[:], cs[:], mybir.ActivationFunctionType.Square,
                accum_out=csq[:])
            # v[p] = (2/f)*csq - (2N/f)*sumsq
            a_sb = sp.tile([P, 1], mybir.dt.float32)
            nc.vector.tensor_scalar_mul(
                a_sb[:], sumsq_p[:], float(2.0 * N_CLASSES / FACTOR))
            v_sb = sp.tile([P, 1], mybir.dt.float32)
            nc.vector.scalar_tensor_tensor(
                out=v_sb[:], in0=csq[:], scalar=float(2.0 / FACTOR),
                in1=a_sb[:], op0=mybir.AluOpType.mult,
                op1=mybir.AluOpType.subtract)
            # collapse partitions: c_k = ones^T v  -> PSUM [1,1]
            ck_ps = ps.tile([1, 1], mybir.dt.float32)
            nc.tensor.matmul(ck_ps[:], lhsT=v_sb[:], rhs=ones_f[:],
                             start=True, stop=True)
            # place own scalar in recv column 0 (partition 0)
            nc.vector.tensor_copy(bass.AP(recv, 0, [[8, 1], [1, 1]]),
                                  ck_ps[:])

    # ---- cross-core sum: single-shot all-to-all of the 8 scalars ----
    # Two waves of 7 single-slot broadcasts (slot d -> core ^ d). Wave 2
    # is a flush: its remote-sem updates trail wave 1's data by a full
    # wire round trip per DMA-engine FIFO, closing the window where a
    # remote sem becomes visible before the SBUF bytes do. Batches of 7
    # stay within the SWDGE broadcast-group FIFO.
    with nc.Block("a2a"):
        def bcol(d):
            return bass.AP(recvb, d, [[8, P], [1, 1]])
        for wave, tgt in enumerate((rcol, bcol)):
            for d in range(1, 8):
                rd = [None] * 8
                rd[d] = (0, d)
                nc.gpsimd.remote_dma_broadcast(
                    tgt(d), rcol(0), remote_sem=dsem, local_sem=lsem,
                    rdests=rd).then_inc(prep_sem, 1)
            nc.gpsimd.wait_ge(prep_sem, 7 * (wave + 1))
            nc.gpsimd.trigger_dma(7)
        nc.vector.wait_ge(dsem, 28)
        # settle margin on top of the flush round trip
        nc.vector.tensor_copy(bass.AP(pad, 0, [[512, P], [1, 512]]),
                              bass.AP(pad, 0, [[512, P], [1, 512]]))
        nc.vector.tensor_add(bass.AP(radd, 0, [[4, P], [1, 4]]),
                             bass.AP(recv, 0, [[8, P], [1, 4]]),
                             bass.AP(recv, 4, [[8, P], [1, 4]])
                             ).then_inc(cp_sem, 1)
        nc.vector.wait_ge(cp_sem, 1)
        nc.vector.tensor_add(bass.AP(radd, 0, [[4, P], [1, 2]]),
                             bass.AP(radd, 0, [[4, P], [1, 2]]),
                             bass.AP(radd, 2, [[4, P], [1, 2]])
                             ).then_inc(cp_sem, 1)
        nc.vector.wait_ge(cp_sem, 2)
        nc.vector.tensor_add(bass.AP(tot, 0, [[1, P], [1, 1]]),
                             bass.AP(radd, 0, [[4, P], [1, 1]]),
                             bass.AP(radd, 1, [[4, P], [1, 1]])
                             ).then_inc(cp_sem, 1)
        nc.sync.wait_ge(cp_sem, 3)
        nc.sync.dma_start(out_ext[:], bass.AP(tot, 0, [[1, 1], [1, 1]])
                          ).then_inc(out_sem, 16)

    nc.finalize()
    es.close()
    return nc


_NC_CACHE = None


def _get_nc():
    global _NC_CACHE
    if _NC_CACHE is None:
        _NC_CACHE = _build()
    return _NC_CACHE


def _run(anchors: np.ndarray, trace: bool = False):
    """Returns (loss_scalar, BassKernelResults)."""
    x = np.asarray(anchors, dtype=np.float32).reshape(N_CLASSES, D)
    in_maps = [
        {"anchors": np.ascontiguousarray(x[:, i * COLS:(i + 1) * COLS])}
        for i in range(N_CORES)
    ]
    nc = _get_nc()
    res = run_bass_kernel_spmd(nc, in_maps, core_ids=list(range(N_CORES)),
                               trace=trace)
    loss = np.float32(np.asarray(res.results[0]["out"]).reshape(())[()])
    return loss, res


def kernel(anchors: np.ndarray) -> np.ndarray:
    loss, _ = _run(anchors)
    return np.asarray(loss, dtype=np.float32).reshape(())


# revision 19
# speedup vs baseline: 2.3822x; 1.0179x over previous
"""AnchorLoss distributed Trainium2 kernel (8 NeuronCores).

reference math (anchors: [8192, 8, 512] f32):
    x = anchors.reshape(8192, 4096)
    loss = -(2*N*sum(x*x) - 2*sum(colsum(x)^2)) / sqrt(512)

Strategy: shard COLUMNS across the 8 cores (512 columns each); every
cross-core quantity is then a single scalar per core. Each core streams
its [8192, 512] f32 slice as 16 row-tiles of [128, 4, 512]:

  - 11 "gram" tiles are cast f32->fp8e4 inside the Pool SWDGE DMA (HBM
    still reads every f32 byte once); PE accumulates per-chunk Gram
    matrices X_c^T X_c into one PSUM bank [128, 4x128] whose diagonal
    is the tile's sum of squares. PE also column-sums every tile via
    ones-vector matmuls into a second PSUM bank [128, 4].
  - 2 tiles are cast f32->bf16 (Pool DMA) and squared on DVE
    (2x tensor_mul + 4x tensor_scalar accumulate).
  - 3 tiles stay f32 on the SP HWDGE ring and are squared on ScalarE.

This splits the elementwise-square roofline across PE/DVE/ACT while
Pool+SP share the DMA bytes, so all five engines run ~balanced.

The per-core partial  c_k = (2/f)*||colsum_k||^2 - (2N/f)*sumsq_k  is
collapsed to a scalar with a ones^T matmul, then summed across cores
WITHOUT the 15us collective: a raw post-tile block runs a single-shot
all-to-all of the 8 scalars via XOR-relative remote_dma_broadcast
(7 single-slot broadcasts, slot d targets core ^ d; the hardware XORs
physical ids, which relabels peers but stays a bijection, so the sum
is invariant). Each core then tree-adds the 8 values and SP DMAs the
total to "out"; the host reads core 0.
"""

import numpy as np
from contextlib import ExitStack

from concourse import bacc, bass, tile, mybir
from concourse.bass_utils import run_bass_kernel_spmd

# The axon client container has no /dev/neuron*, so the driver ioctls
# behind these routing lookups fail. The simulator only needs a sane
# single-device identity mapping (8 cores on device 0); the real NEFF
# resolves XOR-relative routing on-device and never reads these.
import concourse.libnrt as _lnrt
import concourse.bass_interp as _bi
try:
    _lnrt.get_trn2_nc_mapping()
except Exception:
    _IDENT = {(0, i): i for i in range(8)}
    _RID = {0: 0}
    _lnrt.get_trn2_nc_mapping = lambda: _IDENT
    _lnrt.get_device_id_to_routing_id_mapping = lambda: _RID
    _bi.get_device_id_to_routing_id_mapping = lambda: _RID

N_CORES = 8
N_CLASSES = 8192
D = 4096                          # 8 * 512 flattened embedding dim
COLS = D // N_CORES               # 512 columns per core
P = 128                           # partitions
RB = 4                            # row-blocks per tile
TILE_ROWS = P * RB                # 512 rows per tile
N_TILES = N_CLASSES // TILE_ROWS  # 16
CHUNK = 128
N_CHUNKS = COLS // CHUNK          # 4
FACTOR = float(np.sqrt(np.float32(512.0)))

N_GRAM = 11                       # fp8 tiles -> PE gram diag
N_DVE = 2                         # bf16 tiles -> DVE squares
N_ACT = N_TILES - N_GRAM - N_DVE  # f32 tiles (SP DMA) -> ACT squares
PE_WARMUP = 20                    # dummy matmuls to ramp the PE p-state


def _build():
    nc = bacc.Bacc(None, num_devices=N_CORES)
    x_ext = nc.declare_dram_parameter(
        "anchors", [N_CLASSES, COLS], mybir.dt.float32, isOutput=False
    )
    out_ext = nc.declare_dram_parameter(
        "out", [1, 1], mybir.dt.float32, isOutput=True
    )

    es = ExitStack()
    # raw SBUF tensors shared with the post-tile all-to-all block
    recv = es.enter_context(nc.sbuf_tensor("recv8", [P, 8], mybir.dt.float32))
    recvb = es.enter_context(nc.sbuf_tensor("recvb8", [P, 8], mybir.dt.float32))
    radd = es.enter_context(nc.sbuf_tensor("radd", [P, 4], mybir.dt.float32))
    tot = es.enter_context(nc.sbuf_tensor("tot", [P, 1], mybir.dt.float32))
    pad = es.enter_context(nc.sbuf_tensor("pad", [P, 512], mybir.dt.float32))
    lsem = nc.alloc_semaphore("aa_lsem")
    prep_sem = nc.alloc_semaphore("aa_prep")
    dsem = nc.alloc_semaphore("aa_dsem")
    cp_sem = nc.alloc_semaphore("aa_cp")
    out_sem = nc.alloc_semaphore("aa_out")

    def rcol(d):
        return bass.AP(recv, d, [[8, P], [1, 1]])

    with tile.TileContext(nc) as tc:
        with (
            tc.tile_pool(name="io", bufs=6) as io,
            tc.tile_pool(name="small", bufs=1) as sp,
            tc.tile_pool(name="psum", bufs=1, space="PSUM") as ps,
        ):
            # constants (keep Pool free: build on DVE where possible)
            ones8 = sp.tile([P, 1], mybir.dt.float8e4)
            nc.vector.memset(ones8[:], 1.0)
            ones_bf = sp.tile([P, 1], mybir.dt.bfloat16)
            nc.vector.memset(ones_bf[:], 1.0)
            ones_f = sp.tile([P, 1], mybir.dt.float32)
            nc.vector.memset(ones_f[:], 1.0)
            wones = sp.tile([P, CHUNK], mybir.dt.bfloat16)
            nc.vector.memset(wones[:], 0.001)
            nc.vector.memset(bass.AP(recv, 0, [[8, P], [1, 8]]), 0.0)
            nc.vector.memset(bass.AP(pad, 0, [[512, P], [1, 512]]), 0.0)
            # identity mask for the gram diagonal: eye[p, q] = (q == p)
            iq = sp.tile([P, CHUNK], mybir.dt.float32)
            nc.gpsimd.iota(iq[:], [[1, CHUNK]], channel_multiplier=0,
                           allow_small_or_imprecise_dtypes=True)
            ip = sp.tile([P, 1], mybir.dt.float32)
            nc.gpsimd.iota(ip[:], [[0, 1]], channel_multiplier=1,
                           allow_small_or_imprecise_dtypes=True)
            eye = sp.tile([P, CHUNK], mybir.dt.float32)
            nc.vector.tensor_tensor(
                eye[:], iq[:], ip[:].broadcast_to([P, CHUNK]),
                mybir.AluOpType.is_equal)


            # ACT table preload: tiny square so LoadActFuncSet runs early
            warm_a = sp.tile([P, 1], mybir.dt.float32)
            nc.scalar.activation(warm_a[:], ones_f[:],
                                 mybir.ActivationFunctionType.Square)

            # PE p-state warmup: dummy matmuls while DMAs stream
            warm_ps = ps.tile([P, CHUNK], mybir.dt.float32)
            for i in range(PE_WARMUP):
                nc.tensor.matmul(warm_ps[:], lhsT=wones[:], rhs=wones[:],
                                 start=True, stop=True)

            # PSUM accumulators. All 4 column-chunks of every gram tile
            # accumulate into ONE [128,128] bank: its diagonal is then
            # sum_c ||col_{c,q}||^2, i.e. exactly the per-q partial sums
            # of squares (the off-diagonal cross terms are never read).
            gramA = ps.tile([P, CHUNK], mybir.dt.float32, name="gramA")
            gramB = ps.tile([P, CHUNK], mybir.dt.float32, name="gramB")
            cs = ps.tile([P, N_CHUNKS], mybir.dt.float32)

            # accumulator columns for DVE/ACT pieces' row-sums of squares
            rowsumsq = sp.tile([P, 8], mybir.dt.float32)
            nc.vector.memset(rowsumsq[:], 0.0)

            # Tile pieces in per-queue issue order. Pool streams gram
            # tiles fp8 with one bf16 (DVE-squared) tile mid-stream and
            # the last bf16 tile as two tail halves; SP streams 3 f32
            # tiles for ACT. Pieces: (kind, tile_idx, rb_lo, rb_hi).
            # Pool order: 5 gram tiles, both bf16 tiles mid-stream (DVE
            # squares them while streaming), then the remaining gram
            # tiles with the last one as two halves (cheap tail: a half
            # gram is ~0.45us of PE work). Gram bank A covers the first
            # 9 gram tiles and stops early so its diagonal extraction
            # overlaps the stream; bank B covers the last ~2.
            pool_q = []
            sp_q = []
            g_ids = list(range(N_GRAM))                  # tiles 0..10
            v1, v2 = N_GRAM, N_GRAM + 1                  # tiles 11, 12
            a_ids = [N_GRAM + 2, N_GRAM + 3, N_GRAM + 4]  # tiles 13..15
            for i, g in enumerate(g_ids[:5]):
                pool_q.append(("g", g, 0, RB))
            pool_q.append(("v", v1, 0, RB))
            pool_q.append(("v", v2, 0, RB))
            for g in g_ids[5:-1]:
                pool_q.append(("g", g, 0, RB))
            pool_q.append(("g", g_ids[-1], 0, 2))
            pool_q.append(("g", g_ids[-1], 2, RB))
            for a in a_ids:
                sp_q.append(("a", a, 0, RB))
            N_BANK_A = 9                                 # gram tiles in bank A

            # arrival-time estimate to order the consumer-side program
            POOL_D, SP_D = 1883.0, 1717.0
            BYTE_NS = 0.3855

            def piece_bytes(kind, nrb):
                per = {"g": 1, "v": 2, "a": 4}[kind]
                return nrb * COLS * per

            merged = []
            t = 100.0
            for pc in pool_q:
                t += piece_bytes(pc[0], pc[3] - pc[2]) * BYTE_NS
                merged.append((t + POOL_D, pc))
            t = 100.0
            for pc in sp_q:
                t += piece_bytes(pc[0], pc[3] - pc[2]) * BYTE_NS
                merged.append((t + SP_D, pc))
            merged.sort(key=lambda m: m[0])

            bank_a_tiles = set(g_ids[:N_BANK_A])
            n_gram_mm_a = sum(N_CHUNKS * (pc[3] - pc[2]) for _, pc in merged
                              if pc[0] == "g" and pc[1] in bank_a_tiles)
            n_gram_mm_b = sum(N_CHUNKS * (pc[3] - pc[2]) for _, pc in merged
                              if pc[0] == "g" and pc[1] not in bank_a_tiles)
            n_cs_mm = sum(N_CHUNKS * (pc[3] - pc[2]) for _, pc in merged)

            sq_col = 0
            gram_mm_a = 0
            gram_mm_b = 0
            cs_mm = 0
            for _, (kind, ti, rb_lo, rb_hi) in merged:
                nrb = rb_hi - rb_lo
                src = x_ext[ti * TILE_ROWS + rb_lo * P:
                            ti * TILE_ROWS + rb_hi * P, :]
                src = src.rearrange("(rb p) c -> p rb c", rb=nrb, p=P)
                if kind == "g":
                    xt = io.tile([P, nrb, COLS], mybir.dt.float8e4,
                                 tag="xg", name=f"xg{ti}_{rb_lo}")
                    nc.gpsimd.dma_start(xt[:], src)
                    one_t = ones8
                elif kind == "v":
                    xt = io.tile([P, nrb, COLS], mybir.dt.bfloat16,
                                 tag=f"xv{nrb}", name=f"xv{ti}_{rb_lo}",
                                 bufs=2)
                    nc.gpsimd.dma_start(xt[:], src)
                    one_t = ones_bf
                else:
                    xt = io.tile([P, nrb, COLS], mybir.dt.float32,
                                 tag="xa", name=f"xa{ti}_{rb_lo}", bufs=2)
                    nc.sync.dma_start(xt[:], src)
                    one_t = ones_f

                # column sums: cs[m, c] += sum_{p,rb} xt[p, rb, c*128+m]
                for c in range(N_CHUNKS):
                    for j in range(nrb):
                        cs_mm += 1
                        nc.tensor.matmul(
                            cs[:, c:c + 1],
                            lhsT=xt[:, j, c * CHUNK:(c + 1) * CHUNK],
                            rhs=one_t[:],
                            start=(cs_mm == 1), stop=(cs_mm == n_cs_mm),
                        )

                if kind == "g":
                    in_a = ti in bank_a_tiles
                    bank = gramA if in_a else gramB
                    for c in range(N_CHUNKS):
                        for j in range(nrb):
                            if in_a:
                                gram_mm_a += 1
                                st = gram_mm_a == 1
                                sp_ = gram_mm_a == n_gram_mm_a
                            else:
                                gram_mm_b += 1
                                st = gram_mm_b == 1
                                sp_ = gram_mm_b == n_gram_mm_b
                            nc.tensor.matmul(
                                bank[:],
                                lhsT=xt[:, j, c * CHUNK:(c + 1) * CHUNK],
                                rhs=xt[:, j, c * CHUNK:(c + 1) * CHUNK],
                                start=st, stop=sp_,
                            )
                elif kind == "v":
                    scr_vb = io.tile([P, nrb, COLS], mybir.dt.bfloat16,
                                     tag=f"scrv{nrb}", name=f"sv{ti}_{rb_lo}",
                                     bufs=2)
                    nc.vector.tensor_mul(scr_vb[:], xt[:], xt[:])
                    nc.vector.tensor_scalar(
                        scr_vb[:], scr_vb[:], 1.0, None,
                        mybir.AluOpType.mult, mybir.AluOpType.add,
                        accum_out=rowsumsq[:, sq_col:sq_col + 1],
                    )
                    sq_col += 1
                else:
                    scr_a = io.tile([P, nrb, COLS], mybir.dt.bfloat16,
                                    tag="scra", name=f"sa{ti}_{rb_lo}",
                                    bufs=2)
                    nc.scalar.activation(
                        scr_a[:], xt[:],
                        mybir.ActivationFunctionType.Square,
                        accum_out=rowsumsq[:, sq_col:sq_col + 1],
                    )
                    sq_col += 1
            assert sq_col <= 8

            # ---- local tail ----
            # gram diagonals -> per-partition gram sums of squares.
            # Bank A closes mid-stream, so its extraction overlaps the
            # remaining DMAs; only bank B's extraction trails the stream.
            gdA = sp.tile([P, CHUNK], mybir.dt.float32)
            nc.vector.tensor_mul(gdA[:], gramA[:], eye[:])
            sumsq_ga = sp.tile([P, 1], mybir.dt.float32)
            nc.vector.tensor_scalar(
                gdA[:], gdA[:], 1.0, None,
                mybir.AluOpType.mult, mybir.AluOpType.add,
                accum_out=sumsq_ga[:])
            gdB = sp.tile([P, CHUNK], mybir.dt.float32)
            nc.vector.tensor_mul(gdB[:], gramB[:], eye[:])
            sumsq_gb = sp.tile([P, 1], mybir.dt.float32)
            nc.vector.tensor_scalar(
                gdB[:], gdB[:], 1.0, None,
                mybir.AluOpType.mult, mybir.AluOpType.add,
                accum_out=sumsq_gb[:])
            # + DVE/ACT tile row sums
            sumsq_p = sp.tile([P, 1], mybir.dt.float32)
            nc.vector.tensor_reduce(
                out=sumsq_p[:], in_=rowsumsq[:],
                axis=mybir.AxisListType.X, op=mybir.AluOpType.add)
            nc.vector.tensor_add(sumsq_p[:], sumsq_p[:], sumsq_ga[:])
            nc.vector.tensor_add(sumsq_p[:], sumsq_p[:], sumsq_gb[:])
            # colsum^2 per partition (ACT is idle by now)
            csq_scr = sp.tile([P, N_CHUNKS], mybir.dt.float32)
            csq = sp.tile([P, 1], mybir.dt.float32)
            nc.scalar.activation(
                csq_scr[:], cs[:], mybir.ActivationFunctionType.Square,
                accum_out=csq[:])
            # v[p] = (2/f)*csq - (2N/f)*sumsq
            a_sb = sp.tile([P, 1], mybir.dt.float32)
            nc.vector.tensor_scalar_mul(
                a_sb[:], sumsq_p[:], float(2.0 * N_CLASSES / FACTOR))
            v_sb = sp.tile([P, 1], mybir.dt.float32)
            nc.vector.scalar_tensor_tensor(
                out=v_sb[:], in0=csq[:], scalar=float(2.0 / FACTOR),
                in1=a_sb[:], op0=mybir.AluOpType.mult,
                op1=mybir.AluOpType.subtract)
            # collapse partitions: c_k = ones^T v  -> PSUM [1,1]
            ck_ps = ps.tile([1, 1], mybir.dt.float32)
            nc.tensor.matmul(ck_ps[:], lhsT=v_sb[:], rhs=ones_f[:],
                             start=True, stop=True)
            # place own scalar in recv column 0 (partition 0)
            nc.vector.tensor_copy(bass.AP(recv, 0, [[8, 1], [1, 1]]),
                                  ck_ps[:])

    # ---- cross-core sum: single-shot all-to-all of the 8 scalars ----
    # 7 single-slot broadcasts (slot d -> core ^ d; the hardware XORs
    # physical ids, which relabels peers but stays a bijection, so the
    # sum is invariant). After the sem wait, gpsimd spin-polls partition
    # 0 of every slot: remote sem updates can become visible before the
    # SBUF bytes, but a 4-byte scalar lands atomically, so value!=0 is
    # an exact arrival check (the summands are ~1e9 in magnitude; the
    # memset background is 0). The sim satisfies the polls immediately.
    with nc.Block("a2a"):
        for d in range(1, 8):
            rd = [None] * 8
            rd[d] = (0, d)
            nc.gpsimd.remote_dma_broadcast(
                rcol(d), rcol(0), remote_sem=dsem, local_sem=lsem,
                rdests=rd).then_inc(prep_sem, 1)
        nc.gpsimd.wait_ge(prep_sem, 7)
        nc.gpsimd.trigger_dma(7)
        nc.gpsimd.wait_ge(dsem, 14)
        with (nc.gpsimd.register("aa_rv") as rv,
              nc.gpsimd.register("aa_rc") as rc):
            for d in range(1, 8):
                def _cond(d=d):
                    nc.gpsimd.reg_load(
                        rv,
                        bass.AP(recv, d, [[8, 1], [1, 1]]).bitcast(
                            mybir.dt.int32))
                    nc.gpsimd.reg_alu(rc, rv, 0, mybir.AluOpType.is_equal)
                    return rc
                with nc.gpsimd.While(_cond):
                    pass
        nc.gpsimd.sem_inc(cp_sem, 1)
        nc.vector.wait_ge(cp_sem, 1)
        nc.vector.tensor_add(bass.AP(radd, 0, [[4, P], [1, 4]]),
                             bass.AP(recv, 0, [[8, P], [1, 4]]),
                             bass.AP(recv, 4, [[8, P], [1, 4]])
                             ).then_inc(cp_sem, 1)
        nc.vector.wait_ge(cp_sem, 2)
        nc.vector.tensor_add(bass.AP(radd, 0, [[4, P], [1, 2]]),
                             bass.AP(radd, 0, [[4, P], [1, 2]]),
                             bass.AP(radd, 2, [[4, P], [1, 2]])
                             ).then_inc(cp_sem, 1)
        nc.vector.wait_ge(cp_sem, 3)
        nc.vector.tensor_add(bass.AP(tot, 0, [[1, P], [1, 1]]),
                             bass.AP(radd, 0, [[4, P], [1, 1]]),
                             bass.AP(radd, 1, [[4, P], [1, 1]])
                             ).then_inc(cp_sem, 1)
        nc.sync.wait_ge(cp_sem, 4)
        nc.sync.dma_start(out_ext[:], bass.AP(tot, 0, [[1, 1], [1, 1]])
                          ).then_inc(out_sem, 16)

    nc.finalize()
    es.close()
    return nc


_NC_CACHE = None


def _get_nc():
    global _NC_CACHE
    if _NC_CACHE is None:
        _NC_CACHE = _build()
    return _NC_CACHE


def _run(anchors: np.ndarray, trace: bool = False):
    """Returns (loss_scalar, BassKernelResults)."""
    x = np.asarray(anchors, dtype=np.float32).reshape(N_CLASSES, D)
    in_maps = [
        {"anchors": np.ascontiguousarray(x[:, i * COLS:(i + 1) * COLS])}
        for i in range(N_CORES)
    ]
    nc = _get_nc()
    res = run_bass_kernel_spmd(nc, in_maps, core_ids=list(range(N_CORES)),
                               trace=trace)
    loss = np.float32(np.asarray(res.results[0]["out"]).reshape(())[()])
    return loss, res


def kernel(anchors: np.ndarray) -> np.ndarray:
    loss, _ = _run(anchors)
    return np.asarray(loss, dtype=np.float32).reshape(())
